# revision 2
# baseline (speedup 1.0000x reference)
"""Trainium2 Bass kernel for nn_Net_91164975824989 (v2 — multi-engine).

Math: line-MLP on binary spike vectors collapses to a multilinear
polynomial over per-timestep spike values; we work in the sigma = +/-1
(sign) basis.  Per sample we accumulate 33 monomial time-sums over the
25 LIF steps (9 per-cell sums, 18 within-line pair products, 6 triples)
and project with host-derived parity weights.

Engine split (the whole point of v2 — baseline was DVE-bound at 84%):
  - LIF scan in v-space (v = mem - thresh): v' = beta*v + d - (v > 0),
    d = x - 0.05, v1 = x - 1.  One fused custom DVE op per step.
  - sigma_t = Sign(v_{t+1}) on the Activation engine (idle in v1),
    bf16 output, chunked slabs.
  - pair/triple products: split between DVE (tensor_tensor bf16 at 2x)
    and Pool/GpSimd (scalar_tensor_tensor, idle in v1).
  - time-sums on the TensorEngine: identity-weight matmuls accumulating
    into PSUM, 3 consolidated matmuls per timestep (512/512/32 rows).
  - epilogue: weight-mults on Pool straight out of PSUM, X-axis reduce
    on DVE; PSUM split into two t-halves so half the epilogue overlaps
    the second half of the scan.
"""

import numpy as np

B = 32768
N_CORES = 8
B_CORE = B // N_CORES          # 4096
P = 128                        # partitions
SPP = B_CORE // P              # 32 samples per partition
C = 9                          # cells
T = 25                         # timesteps
NF = 33                        # feature slots
FT = NF * SPP                  # 1056 feature row elements per t
BETA = 0.95
# chunk bounds over t = 1..24 (t=0 spikes are constant -1, folded on host)
# small first chunk -> sigma/products start early; small last -> short tail
TGROUP = (3, 4, 5, 6, 4, 2)
# which product groups run on Pool (rest on DVE). Pool only runs
# tensor_tensor add/sub/mult (is_* and TensorScalarPtr fail the ISA
# engine check), but arbitrary-rank views are fine.
POOL_GROUPS = ("cp03", "ctr")

_STATE: dict = {}

# feature slot map (f-major within the 1056-wide feats row).
# Slots 0..15 (= PSUM region 0) hold only Act- and Pool-produced values,
# so PE can accumulate region 0 while DVE is still inside the scan chain.
SL_SIG = 0     # 9 slots: sigma cell sums                       [Act]
SL_CP03 = 9    # 6: col pairs c..c+3 (c=0..5)                   [Pool]
SL_CP06 = 15   # 3: col pairs j..j+6                            [Pool j=0,1 / DVE j=2]
SL_RP01 = 18   # 6: row pairs (i, a): a=0 -> (01), a=1 -> (12)  [DVE]
SL_RP02 = 24   # 3: row pairs (pos0,pos2)                       [DVE]
SL_RTR = 27    # 3: row triples                                 [DVE]
SL_CTR = 30    # 3: col triples                                 [DVE]


def _host_coeffs(W1, b1, W2, b2, W3, b3, W4, b4):
    """Parity (sigma-basis) coefficients of the line-MLP p1 output ->
    33 slot weights + constant. All float64."""
    W1, b1, W2, b2, W3, b3, W4, b4 = [
        np.asarray(a, np.float64) for a in (W1, b1, W2, b2, W3, b3, W4, b4)
    ]

    def mlp_p1(s):
        h = np.maximum(W1 @ s + b1, 0)
        h = np.maximum(W2 @ h + b2, 0)
        h = np.maximum(W3 @ h + b3, 0)
        h = np.maximum(W4 @ h + b4, 0)
        e = np.exp(h - h.max())
        return e[1] / e.sum()

    u = np.zeros(8)
    for code in range(8):
        u[code] = mlp_p1(np.array([(code >> j) & 1 for j in range(3)], np.float64))

    d = np.zeros(8)
    for S in range(8):
        for code in range(8):
            chi = 1.0
            for j in range(3):
                if (S >> j) & 1:
                    chi *= 2 * ((code >> j) & 1) - 1
            d[S] += u[code] * chi
        d[S] /= 8.0
    d0 = d[0]
    d1 = [d[1], d[2], d[4]]
    d01, d02, d12 = d[3], d[5], d[6]
    d123 = d[7]

    w = np.zeros(NF)
    for c in range(9):
        i, j = divmod(c, 3)
        w[SL_SIG + c] = d1[j] + d1[i]
    w[SL_CP03:SL_CP03 + 3] = d01
    w[SL_CP03 + 3:SL_CP03 + 6] = d12
    w[SL_CTR:SL_CTR + 3] = d123
    w[SL_RP01:SL_RP01 + 6] = [d01, d12] * 3
    w[SL_RP02:SL_RP02 + 3] = d02
    w[SL_RTR:SL_RTR + 3] = d123
    w[SL_CP06:SL_CP06 + 3] = d02

    # device sums cover t = 1..24; t=0 is the all-minus pattern
    k1 = 144.0 * d0 + 6.0 * u[0]
    return w, k1


def _register_vstep_op():
    """Custom fused DVE op: out = s0*in0 + in1 - (in0 > 0)."""
    import re
    from concourse import dve_ops
    from concourse.dve_spec import Spec, Src0, Src1, C0, Zero

    for o in dve_ops.OPS:
        if o.name == "LIF_VSTEP_ANT":
            return o
    spec = Spec(
        body=Src0 * C0 + Src1 - (Src0 > Zero),
        reference=lambda in0, in1, s0, s1, imm2: in0 * s0 + in1
        - (in0 > 0).astype(in0.dtype),
    )
    op = dve_ops.DveOp("LIF_VSTEP_ANT", spec, subdim=False, uops_sha={})
    dve_ops.OPS.append(op)
    dve_ops.CUSTOM_DVE_SPECS[op.name] = spec
    dve_ops._SUB_OPCODE_FOR_NAME[op.name] = (
        max(dve_ops._SUB_OPCODE_FOR_NAME.values()) + 1)
    for ver in ("v3", "v4"):
        try:
            op.compile(ver)
        except ValueError as e:
            m = re.search(r'\]="([0-9a-f]+)"', str(e))
            if not m:
                raise
            op.uops_sha[ver] = m.group(1)
    return op


def _build_module(tgroup=TGROUP, pool_groups=POOL_GROUPS):
    import concourse.bass as bass
    import concourse.tile as tile
    from concourse import bacc, mybir
    from contextlib import ExitStack

    vstep_op = _register_vstep_op()

    f32 = mybir.dt.float32
    bf16 = mybir.dt.bfloat16
    Alu = mybir.AluOpType
    Act = mybir.ActivationFunctionType

    nc = bacc.Bacc("TRN2", target_bir_lowering=False, debug=False,
                   num_devices=N_CORES)

    # aux blob per partition: [ w expanded f-major: 1056 f32 |
    # consts: 2 | identity row: 128 bf16 = 64 f32 ]
    BLOB = FT + 2 + P // 2
    xs = nc.declare_dram_parameter("xs", [B_CORE, C], f32, isOutput=False)
    blob = nc.declare_dram_parameter("blob", [P, BLOB], f32, isOutput=False)
    y = nc.declare_dram_parameter("y", [B_CORE, 2], f32, isOutput=True)

    # chunk bounds over t in [1, 25)
    bounds = []
    acc = 1
    for g in tgroup:
        bounds.append((acc, acc + g))
        acc += g
    assert acc == T, bounds

    with tile.TileContext(nc) as tc, ExitStack() as ctx:
        pool = ctx.enter_context(tc.tile_pool(name="main", bufs=1))
        psum = ctx.enter_context(tc.tile_pool(name="psum", bufs=1, space="PSUM"))

        x_raw = pool.tile([P, SPP, C], f32)
        xs_r = xs.rearrange("(p s) c -> p s c", p=P)
        H = SPP // 2
        nc.sync.dma_start(x_raw, xs_r)
        blob_sb = pool.tile([P, BLOB], f32)
        # blob on the Activation HWDGE ring: x has the SP ring to itself
        nc.scalar.dma_start(blob_sb, blob[:, :])
        w_sb = blob_sb[:, :FT]                       # [P, 1056] f32
        consts_sb = blob_sb[:, FT:FT + 2]
        id_sb = blob_sb[:, FT + 2:].bitcast(bf16)    # [P, 128]

        dlt = pool.tile([P, C, SPP], f32)           # d = x - 0.05, [p, c, s]
        vh = pool.tile([P, T, C, SPP], f32)         # slot t: v_{t+1} (sigma_t)
        feats = pool.tile([P, T - 1, FT], bf16)     # slot t-1 for t = 1..24

        # ---- prologue on DVE (in-order with the scan: no cross-engine sem)
        # v1 = x-1 < 0 always (sigma_0 = -1, folded on host), so
        # v2 = 0.95*v1 + d = 1.95x - 1 directly; the scan starts at k=2.
        # Per-half: each scan half-chain starts as soon as its x half lands.
        for h in (slice(0, H), slice(H, SPP)):
            nc.vector.tensor_scalar(
                out=dlt[:, :, h].rearrange("p c s -> p s c"), in0=x_raw[:, h],
                scalar1=1.0, scalar2=-0.05, op0=Alu.mult, op1=Alu.add)
            nc.vector.tensor_scalar(
                out=vh[:, 1, :, h].rearrange("p c s -> p s c"),
                in0=x_raw[:, h],
                scalar1=1.95, scalar2=-1.0, op0=Alu.mult, op1=Alu.add)

        # views over the sigma region of feats
        sg = feats[:, :, :C * SPP].rearrange("p t (c s) -> p t c s", c=C)
        sg_r = feats[:, :, :C * SPP].rearrange("p t (i j s) -> p t i j s",
                                               i=3, j=3)

        def fslot(s0, n):
            return feats[:, :, s0 * SPP:(s0 + n) * SPP].rearrange(
                "p t (f s) -> p t f s", f=n)

        rp01_v = feats[:, :, SL_RP01 * SPP:(SL_RP01 + 6) * SPP].rearrange(
            "p t (i a s) -> p t i a s", i=3, a=2)
        rp02_v = fslot(SL_RP02, 3)
        rtr_v = fslot(SL_RTR, 3)
        cp03_v = fslot(SL_CP03, 6)
        cp06_v = fslot(SL_CP06, 3)
        ctr_v = fslot(SL_CTR, 3)

        # PSUM accumulators: three regions split by producer so each
        # region's matmuls and epilogue fire as soon as its inputs exist
        # (sigma: Act-paced; pool products; DVE products)
        RG = ((0, 288), (288, 576), (576, 1056))
        ps = [psum.tile([P, b - a], f32, name=f"ps_{i}")
              for i, (a, b) in enumerate(RG)]
        fm = pool.tile([P, FT], f32)
        redp = pool.tile([P, len(RG), SPP], f32)
        red = pool.tile([P, SPP], f32)

        def emit_sigma(t0, t1):
            # sigma_t = Sign(v_{t+1}); vh slot t, feats slot t-1 (bf16)
            nc.scalar.activation(
                out=feats[:, t0 - 1:t1 - 1, :C * SPP],
                in_=vh[:, t0:t1].rearrange("p t c s -> p t (c s)"),
                func=Act.Sign)

        # flat 3D views (contiguous slot ranges)
        sgf = feats[:, :, :C * SPP]

        def flat(s0, n):
            return feats[:, :, s0 * SPP:(s0 + n) * SPP]

        def prod_op(eng, out, in0, in1):
            if eng == "dve":
                nc.vector.tensor_mul(out, in0, in1)
            else:
                nc.gpsimd.tensor_tensor(out=out, in0=in0, in1=in1,
                                        op=Alu.mult)

        def emit_products_pool(t0, t1):
            # col pairs on Pool (plain mult, sigma in +/-1)
            ts = slice(t0 - 1, t1 - 1)
            nc.gpsimd.tensor_tensor(
                out=flat(SL_CP03, 6)[:, ts], in0=sgf[:, ts, 0:6 * SPP],
                in1=sgf[:, ts, 3 * SPP:9 * SPP], op=Alu.mult)
            nc.gpsimd.tensor_tensor(
                out=flat(SL_CP06, 3)[:, ts], in0=sgf[:, ts, 0:3 * SPP],
                in1=sgf[:, ts, 6 * SPP:9 * SPP], op=Alu.mult)

        def emit_products_dve_a(t0, t1):
            ts = slice(t0 - 1, t1 - 1)
            nc.vector.tensor_mul(rp01_v[:, ts], sg_r[:, ts, :, 0:2],
                                 sg_r[:, ts, :, 1:3])
            nc.vector.tensor_mul(rp02_v[:, ts], sg_r[:, ts, :, 0],
                                 sg_r[:, ts, :, 2])

        def emit_products_dve_b(t0, t1):
            ts = slice(t0 - 1, t1 - 1)
            nc.vector.tensor_mul(rtr_v[:, ts], rp01_v[:, ts, :, 0],
                                 sg_r[:, ts, :, 2])
            nc.vector.tensor_mul(ctr_v[:, ts], cp03_v[:, ts, 0:3],
                                 sg[:, ts, 6:9])

        def emit_matmuls_region(r, t0, t1):
            a, b = RG[r]
            for t in range(t0, t1):
                nc.tensor.matmul(ps[r][:], id_sb, feats[:, t - 1, a:b],
                                 start=(t == 1), stop=(t == T - 1),
                                 skip_group_check=True)

        def emit_epilogue_region(r):
            # fm = psum * w straight out of PSUM on DVE + partial X-reduce
            a, b = RG[r]
            w_slc = slice(a, b)
            nc.vector.tensor_mul(fm[:, w_slc], ps[r][:], w_sb[:, w_slc])
            if b - a > SPP:
                nc.vector.tensor_reduce(
                    out=redp[:, r],
                    in_=fm[:, w_slc].rearrange("p (f s) -> p s f", s=SPP),
                    axis=mybir.AxisListType.X, op=Alu.add)
            else:
                nc.vector.tensor_copy(out=redp[:, r], in_=fm[:, w_slc])

        def emit_final_reduce():
            nc.vector.tensor_reduce(
                out=red, in_=redp.rearrange("p r s -> p s r"),
                axis=mybir.AxisListType.X, op=Alu.add)

        # ---- software-pipelined emission ----
        # DVE runs the pure scan chain first (it is the serial critical
        # path; its products would steal the ack gaps via the OOO window
        # and push the chain out).  Act's sigma, Pool's products and PE's
        # region-0 matmuls all flow during the scan; DVE's products, the
        # region-1/2 matmuls and the epilogue follow after.
        # scan as two independent s-half chains: halves interleave on the
        # DVE so the chain advances every 2x210 ns (engine-bound) instead
        # of 455 ns (dependency-latency-bound)
        HH = SPP // 2
        NCH = len(bounds)
        for ci in range(NCH):
            for k in range(bounds[ci][0], bounds[ci][1]):
                if k < 2:
                    continue  # vh[1] comes from the prologue
                for h in (slice(0, HH), slice(HH, SPP)):
                    nc.vector._custom_dve(vstep_op, out=vh[:, k, :, h],
                                          in0=vh[:, k - 1, :, h],
                                          in1=dlt[:, :, h], s0=BETA)
            emit_sigma(*bounds[ci])
            emit_matmuls_region(0, *bounds[ci])      # sigma-paced
            emit_products_pool(*bounds[ci])
            if ci >= 2:
                # pool runs ~2 chunks behind the scan; delay its region's
                # matmuls accordingly so they don't block the PE queue
                emit_matmuls_region(1, *bounds[ci - 2])
        for cj in range(NCH):
            emit_products_dve_a(*bounds[cj])
            emit_products_dve_b(*bounds[cj])
            emit_matmuls_region(2, *bounds[cj])
        # the last pool-paced region-1 matmuls go after the DVE-fed
        # stream so PE clears that first
        emit_matmuls_region(1, *bounds[NCH - 2])
        emit_matmuls_region(1, *bounds[NCH - 1])
        emit_epilogue_region(0)
        emit_epilogue_region(2)
        emit_epilogue_region(1)
        emit_final_reduce()

        # ---- final combine ----
        out_t = pool.tile([P, SPP, 2], f32)
        # out1 = red + k1 ; out0 = (150 - k1) - red
        nc.vector.tensor_single_scalar(
            out=out_t[:, :, 1], in_=red, scalar=consts_sb[:, 0:1], op=Alu.add)
        nc.vector.tensor_scalar(
            out=out_t[:, :, 0], in0=red, scalar1=-1.0,
            scalar2=consts_sb[:, 1:2], op0=Alu.mult, op1=Alu.add)
        nc.sync.dma_start(y.rearrange("(p s) o -> p s o", p=P), out_t)

    nc.compile()
    return nc


def _get_module():
    if "nc" not in _STATE:
        _STATE["nc"] = _build_module()
    return _STATE["nc"]


def kernel(x, W1, b1, W2, b2, W3, b3, W4, b4, _trace=False):
    import ml_dtypes
    from concourse.bass_utils import run_bass_kernel_spmd

    w33, k1 = _host_coeffs(W1, b1, W2, b2, W3, b3, W4, b4)

    xs = np.asarray(x, np.float32).reshape(N_CORES, P, SPP * C)
    wrow = np.concatenate([np.repeat(w33, SPP),
                           [k1, 150.0 - k1]]).astype(np.float32)
    wk = np.tile(wrow[None, :], (P, 1))                      # [P, 1058]
    ident_f32 = np.ascontiguousarray(
        np.eye(P, dtype=ml_dtypes.bfloat16)).view(np.float32)  # [P, 64]

    nc = _get_module()
    blob = np.ascontiguousarray(np.concatenate([wk, ident_f32], axis=1))
    in_maps = [{"xs": np.ascontiguousarray(xs[i].reshape(B_CORE, C)),
                "blob": blob} for i in range(N_CORES)]
    res = run_bass_kernel_spmd(nc, in_maps, core_ids=list(range(N_CORES)),
                               trace=_trace)
    out = np.concatenate([res.results[i]["y"] for i in range(N_CORES)], axis=0)
    if _trace:
        _STATE["last_results"] = res
    return out.astype(np.float32)
